# revision 32
# baseline (speedup 1.0000x reference)
"""Trainium2 Bass kernel for NnqlmCnnBasedLstm.

Math (per batch item, per input sequence q/a):
  xe = embed[idx]                      (L, D)       D = 128
  dens_t = outer(xe_t, xe_t)/(|xe_t|^2 + 1e-4)     (D, D), symmetric
  2-layer ConvLSTM over L=40 steps; each gate g:
    pre_g = conv2d([xt; h], W_g, stride=(2,1), pad=(1,1)) + b_g  on (2D, D) -> (D, D)
  c = sig(f)*c + sig(i)*tanh(cc); h = sig(o)*tanh(c)
  out = max_t h2_t  -> flatten -> concat(q,a) -> linear(2) -> log_softmax

Device strategy (8 cores, data parallel over B=32 -> 4 items/core, each with a
q-chain and an a-chain = 8 chains/core):
  * State kept TRANSPOSED: tiles are (w partitions, j free).  The density is
    symmetric so layer-1 inputs need no transpose.
  * conv: out_T[w, j] = sum_{dh,dw} W[dh,dw] * inp_T[w-1+dw, 2j-1+dh].
    For each dh this is a 3-diagonal Toeplitz band matrix (over w) applied via
    the TensorEngine, with the (2j-1+dh) selection expressed as a stride-2
    free-axis access pattern on the moving operand.  The 4 dh taps are
    processed as 2 DoubleRow fp8 passes (two (weights, ifmap) planes
    accumulated per pass at 0.5 cycles/row) -> 4x fewer PE cycles than bf16.
  * Densities are computed on host (fp8) and DMA'd straight into the x-slots
    of the layer-1 input tiles, one step per transfer.
  * sigmoid/tanh (+conv bias) on ScalarE reading PSUM, outputs bf16; cell
    state and cell-update DVE ops in bf16 (4x DVE mode); h written to the fp8
    input tiles of the consuming layer.
  * Layers are software-pipelined with a 1-step skew (L1 step t runs
    interleaved with L2 step t-1) so the tensor engine always has the other
    layer's matmuls while one layer's act/vector trail completes.
  * Embedding gather, final linear + log_softmax on host (tiny).
"""

import os
import sys

import numpy as np

for _p in ("/opt/trn_rl_repo", "/root/.axon_site/_ro/trn_rl_repo"):
    if os.path.isdir(_p) and _p not in sys.path:
        sys.path.insert(0, _p)

B, L, D, V, NL = 32, 40, 128, 32000, 2
NCORES = 8
CH = 8            # chains per core: 4 batch items x {q, a}
SEG = 2 * D + 2   # per-chain column span in the input tile: [0]=0, [1..128]=x, [129..256]=h, [257]=0
NF = CH * SEG

_CACHE = {}


def _build_nc(L=L):
    import concourse.bass as bass
    import concourse.bacc as bacc
    import concourse.mybir as mybir
    from concourse import tile

    f32 = mybir.dt.float32
    bf16 = mybir.dt.bfloat16
    f8 = mybir.dt.float8e4
    AF = mybir.ActivationFunctionType
    ALU = mybir.AluOpType
    DR = mybir.MatmulPerfMode.DoubleRow

    nc = bacc.Bacc(None, target_bir_lowering=False)

    dens_d = nc.dram_tensor("dens", (L, D, CH, D), f8, kind="ExternalInput")
    st_d = nc.dram_tensor("st", (D, NL * 4 * 2 * 2 * D), f8, kind="ExternalInput")
    bias_d = nc.dram_tensor("bias", (D, NL * 4), f32, kind="ExternalInput")
    out_d = nc.dram_tensor("mp_out", (D, CH * D), bf16, kind="ExternalOutput")

    with tile.TileContext(nc) as tc:
        with (
            tc.tile_pool(name="const", bufs=1) as constp,
            tc.tile_pool(name="state", bufs=1) as statep,
            tc.tile_pool(name="inp0", bufs=3) as inp0p,
            tc.tile_pool(name="inp1", bufs=2) as inp1p,
            tc.tile_pool(name="stg", bufs=3) as stgp,
            tc.tile_pool(name="gate", bufs=2) as gatep,
            tc.tile_pool(name="psum", bufs=1, space="PSUM") as psump,
        ):
            # ---- constants ----
            stT = constp.tile([D, NL * 4 * 2 * 2 * D], f8, tag="stT")
            nc.sync.dma_start(stT[:], st_d[:])

            bias = constp.tile([D, NL * 4], f32, tag="bias")
            nc.sync.dma_start(bias[:], bias_d[:])

            # ---- persistent state ----
            c_l = [statep.tile([D, CH * D], bf16, tag=f"c{l}", name=f"c{l}") for l in range(NL)]
            mp = statep.tile([D, CH * D], bf16, tag="mp")
            for l in range(NL):
                nc.vector.memset(c_l[l][:], 0.0)
            nc.vector.memset(mp[:], -1e30)

            def seg3(t):  # (p, s, c) view of an input tile
                return t[:].rearrange("p (s c) -> p s c", s=CH)

            def new_inp(pool, tag):
                t = pool.tile([D, NF], f8, tag=tag, name=tag)
                # zero the pad columns (0 and 257 of each chain segment)
                v = seg3(t)
                nc.vector.memset(v[:, :, 0:1], 0.0)
                nc.vector.memset(v[:, :, SEG - 1:SEG], 0.0)
                return t

            def load_dens(t_idx, dst):
                """DMA the step-t density into a contiguous staging tile,
                then gpsimd-scatter it into the strided x-slots.  A direct
                strided-dst DMA intermittently raced its completion
                semaphore (NaN poison at 64B burst boundaries)."""
                stg = stgp.tile([D, CH * D], f8, tag="xstg", name="xstg")
                nc.sync.dma_start(stg[:], dens_d[t_idx])
                nc.gpsimd.tensor_copy(
                    seg3(dst)[:, :, 1:1 + D],
                    stg[:].rearrange("p (s r) -> p s r", s=CH),
                )

            GTAG = ["pf", "pi", "po", "pc"]

            def gates(l, inp):
                """DoubleRow gate matmuls + activations + cell update for
                layer l.  Returns (og, th) for the caller to route h."""
                i3 = seg3(inp)
                ps = [psump.tile([D, CH * D], f32, tag=GTAG[g], name=GTAG[g]) for g in range(4)]
                # issue order (f, i, c~, o): the cell-update DVE chain needs
                # f, i, c~ first; o is only needed for the final h product.
                for g in (0, 1, 3, 2):
                    for P in range(2):
                        idx = ((l * 4 + g) * 2 + P) * 2 * D
                        lhsT = stT[:, idx:idx + 2 * D].rearrange(
                            "p (two m) -> p two m", two=2)
                        for half in range(2):
                            rhs = i3[:, half * 4:(half + 1) * 4,
                                     2 * P:2 * P + 2 * D].rearrange(
                                "p s (j two) -> p two s j", two=2)
                            nc.tensor.matmul(
                                ps[g][:, half * 512:(half + 1) * 512],
                                lhsT, rhs,
                                start=(P == 0), stop=(P == 1),
                                perf_mode=DR,
                            )
                fg = gatep.tile([D, CH * D], bf16, tag="fg")
                ig = gatep.tile([D, CH * D], bf16, tag="ig")
                og = gatep.tile([D, CH * D], bf16, tag="og")
                cs = gatep.tile([D, CH * D], bf16, tag="cs")
                nc.scalar.activation(fg[:], ps[0][:], AF.Sigmoid,
                                     bias=bias[:, l * 4 + 0: l * 4 + 1])
                nc.scalar.activation(ig[:], ps[1][:], AF.Sigmoid,
                                     bias=bias[:, l * 4 + 1: l * 4 + 2])
                nc.scalar.activation(cs[:], ps[3][:], AF.Tanh,
                                     bias=bias[:, l * 4 + 3: l * 4 + 4])
                nc.scalar.activation(og[:], ps[2][:], AF.Sigmoid,
                                     bias=bias[:, l * 4 + 2: l * 4 + 3])
                t1 = gatep.tile([D, CH * D], bf16, tag="t1")
                t2 = gatep.tile([D, CH * D], bf16, tag="t2")
                nc.vector.tensor_mul(t1[:], fg[:], c_l[l][:])
                nc.vector.tensor_mul(t2[:], ig[:], cs[:])
                nc.vector.tensor_add(c_l[l][:], t1[:], t2[:])
                th = gatep.tile([D, CH * D], bf16, tag="th")
                nc.scalar.activation(th[:], c_l[l][:], AF.Tanh)
                return og, th

            # software-pipelined schedule with 1-step layer skew.  L1 input
            # tiles are triple-buffered and their density DMA is issued two
            # steps ahead so the transfer has ~2 full phases to land before
            # the matmuls read it.
            i0 = [new_inp(inp0p, "inp0"), new_inp(inp0p, "inp0")]
            nc.vector.memset(seg3(i0[0])[:, :, 129:129 + D], 0.0)  # h1_{-1}=0
            load_dens(0, i0[0])
            load_dens(1, i0[1])
            prevL2 = None                                 # L2 input, t-1

            for t in range(L):
                curL1 = i0[0]
                nxt1 = i0[1] if t + 1 < L else None
                if t + 2 < L:
                    t2 = new_inp(inp0p, "inp0")
                    load_dens(t + 2, t2)
                    i0 = [i0[1], t2]
                else:
                    i0 = [i0[1], None]

                l2in = new_inp(inp1p, "inp1")             # L2 input for step t
                if t == 0:
                    nc.vector.memset(seg3(l2in)[:, :, 129:129 + D], 0.0)

                # --- layer 1, step t ---
                og, th = gates(0, curL1)
                # h1_t -> h-part of L1 input t+1 (needed first, by L1 t+1's
                # matmuls) and x-part of L2 input t (consumed a phase later)
                if nxt1 is not None:
                    nc.vector.tensor_mul(seg3(nxt1)[:, :, 129:129 + D], og[:], th[:])
                nc.vector.tensor_mul(seg3(l2in)[:, :, 1:1 + D], og[:], th[:])

                # --- layer 2, step t-1 ---
                if t >= 1:
                    og, th = gates(1, prevL2)
                    nc.vector.tensor_mul(seg3(l2in)[:, :, 129:129 + D], og[:], th[:])
                    h2b = gatep.tile([D, CH * D], bf16, tag="h2b")
                    nc.vector.tensor_mul(h2b[:], og[:], th[:])
                    nc.vector.tensor_tensor(mp[:], mp[:], h2b[:], op=ALU.max)

                prevL2 = l2in

            # --- layer 2, final step L-1 ---
            og, th = gates(1, prevL2)
            h2b = gatep.tile([D, CH * D], bf16, tag="h2b")
            nc.vector.tensor_mul(h2b[:], og[:], th[:])
            nc.vector.tensor_tensor(mp[:], mp[:], h2b[:], op=ALU.max)

            nc.sync.dma_start(out_d[:], mp[:])

    nc.compile()
    return nc


def _prep_core_inputs(xe_y, st8, bias_arr, core, f8np):
    """xe_y: (B, 2, L, D) sqrt-normalized bf16 embeddings (axis1: 0=q, 1=a)."""
    sl = slice(4 * core, 4 * core + 4)
    # chains: s=0..3 -> q items, s=4..7 -> a items
    v = np.concatenate([xe_y[sl, 0], xe_y[sl, 1]], axis=0).astype(np.float32)  # (8, L, D)
    # density, laid out [t][w][s][r] so the per-step DMA is contiguous
    dens = np.einsum('stw,str->twsr', v, v).astype(f8np)       # (L, D, CH, D)
    return {"dens": dens, "st": st8, "bias": bias_arr}


def kernel(q, a, embed, conv_w, conv_b, lin_w, lin_b):
    import concourse.mybir as mybir
    from concourse import bass_utils
    import ml_dtypes

    bf16 = ml_dtypes.bfloat16
    f8np = mybir.dt.np(mybir.dt.float8e4)

    q = np.asarray(q); a = np.asarray(a)
    embed = np.asarray(embed, np.float32)
    conv_w = np.asarray(conv_w, np.float32)
    conv_b = np.asarray(conv_b, np.float32)
    lin_w = np.asarray(lin_w, np.float32)
    lin_b = np.asarray(lin_b, np.float32)

    # host: embedding gather + density normalization factors
    idx = np.stack([q, a], axis=1).astype(np.int64)            # (B, 2, L)
    xe = embed[idx].astype(np.float64)                         # (B, 2, L, D)
    dot = np.sum(xe * xe, axis=-1, keepdims=True) + 1e-4
    xe_y = (xe / np.sqrt(dot)).astype(np.float32).astype(bf16)

    # host: Toeplitz band stationaries  lhsT[(l,g,dh)] = B^T,
    # B[w, w'] = W[dh, w'-w+1]  (3 diagonals); packed in DoubleRow dh-pairs
    st = np.zeros((NL * 4 * 4, D, D), np.float32)
    for l in range(NL):
        for g in range(4):
            W = conv_w[l, g, 0, 0]                             # (4, 3)
            for dh in range(4):
                Bm = sum(W[dh, dw] * np.eye(D, k=dw - 1) for dw in range(3))
                st[(l * 4 + g) * 4 + dh] = Bm.T.astype(np.float32)
    # pack as one contiguous (D, 16*2*D) block: [w'][pair, plane, m]
    st8 = st.reshape(NL * 4 * 2, 2, D, D).transpose(2, 0, 1, 3).astype(f8np)
    st8 = np.ascontiguousarray(st8).reshape(D, NL * 4 * 2 * 2 * D)
    bias_arr = np.tile(conv_b.reshape(1, -1), (D, 1)).astype(np.float32)

    if "nc" not in _CACHE:
        _CACHE["nc"] = _build_nc()
    nc = _CACHE["nc"]

    in_maps = [_prep_core_inputs(xe_y, st8, bias_arr, i, f8np) for i in range(NCORES)]
    _CACHE["in_maps"] = in_maps
    res = bass_utils.run_bass_kernel_spmd(nc, in_maps, core_ids=list(range(NCORES)))

    # host: unshard + final linear + log_softmax
    q_p = np.zeros((B, D * D), np.float32)
    a_p = np.zeros((B, D * D), np.float32)
    for i in range(NCORES):
        out = np.asarray(res.results[i]["mp_out"], np.float32)  # (D w, CH*D)
        for s in range(CH):
            mp_T = out[:, s * D:(s + 1) * D]                   # (w, j)
            flat = np.ascontiguousarray(mp_T.T).reshape(-1)    # j-major
            if s < 4:
                q_p[4 * i + s] = flat
            else:
                a_p[4 * i + s - 4] = flat
    qa = np.concatenate([q_p, a_p], axis=1)
    score = qa @ lin_w.T + lin_b
    m = score.max(axis=1, keepdims=True)
    ls = score - m
    lse = np.log(np.exp(ls).sum(axis=1, keepdims=True))
    return (ls - lse).astype(np.float32)


# revision 33
# speedup vs baseline: 1.2285x; 1.2285x over previous
"""Trainium2 Bass kernel for NnqlmCnnBasedLstm.

Math (per batch item, per input sequence q/a):
  xe = embed[idx]                      (L, D)       D = 128
  dens_t = outer(xe_t, xe_t)/(|xe_t|^2 + 1e-4)     (D, D), symmetric
  2-layer ConvLSTM over L=40 steps; each gate g:
    pre_g = conv2d([xt; h], W_g, stride=(2,1), pad=(1,1)) + b_g  on (2D, D) -> (D, D)
  c = sig(f)*c + sig(i)*tanh(cc); h = sig(o)*tanh(c)
  out = max_t h2_t  -> flatten -> concat(q,a) -> linear(2) -> log_softmax

Device strategy (8 cores, data parallel over B=32 -> 4 items/core, each with a
q-chain and an a-chain = 8 chains/core):
  * State kept TRANSPOSED: tiles are (w partitions, j free).  The density is
    symmetric so layer-1 inputs need no transpose.
  * conv: out_T[w, j] = sum_{dh,dw} W[dh,dw] * inp_T[w-1+dw, 2j-1+dh].
    For each dh this is a 3-diagonal Toeplitz band matrix (over w) applied via
    the TensorEngine, with the (2j-1+dh) selection expressed as a stride-2
    free-axis access pattern on the moving operand.  The 4 dh taps are
    processed as 2 DoubleRow fp8 passes (two (weights, ifmap) planes
    accumulated per pass at 0.5 cycles/row) -> 4x fewer PE cycles than bf16.
  * Densities are computed on host (fp8) and DMA'd straight into the x-slots
    of the layer-1 input tiles, one step per transfer.
  * sigmoid/tanh (+conv bias) on ScalarE reading PSUM, outputs bf16; cell
    state and cell-update DVE ops in bf16 (4x DVE mode); h written to the fp8
    input tiles of the consuming layer.
  * Layers are software-pipelined with a 1-step skew (L1 step t runs
    interleaved with L2 step t-1) so the tensor engine always has the other
    layer's matmuls while one layer's act/vector trail completes.
  * Embedding gather, final linear + log_softmax on host (tiny).
"""

import os
import sys

import numpy as np

for _p in ("/opt/trn_rl_repo", "/root/.axon_site/_ro/trn_rl_repo"):
    if os.path.isdir(_p) and _p not in sys.path:
        sys.path.insert(0, _p)

B, L, D, V, NL = 32, 40, 128, 32000, 2
NCORES = 8
CH = 8            # chains per core: 4 batch items x {q, a}
SEG = 2 * D + 2   # per-chain column span in the input tile: [0]=0, [1..128]=x, [129..256]=h, [257]=0
NF = CH * SEG

_CACHE = {}


def _build_nc(L=L):
    import concourse.bass as bass
    import concourse.bacc as bacc
    import concourse.mybir as mybir
    from concourse import tile

    f32 = mybir.dt.float32
    bf16 = mybir.dt.bfloat16
    f8 = mybir.dt.float8e4
    AF = mybir.ActivationFunctionType
    ALU = mybir.AluOpType
    DR = mybir.MatmulPerfMode.DoubleRow

    nc = bacc.Bacc(None, target_bir_lowering=False)

    dens_d = nc.dram_tensor("dens", (L, D, CH, D), f8, kind="ExternalInput")
    st_d = nc.dram_tensor("st", (D, NL * 4 * 2 * 2 * D), f8, kind="ExternalInput")
    bias_d = nc.dram_tensor("bias", (D, NL * 4), f32, kind="ExternalInput")
    out_d = nc.dram_tensor("mp_out", (D, CH * D), bf16, kind="ExternalOutput")

    with tile.TileContext(nc) as tc:
        with (
            tc.tile_pool(name="const", bufs=1) as constp,
            tc.tile_pool(name="state", bufs=1) as statep,
            tc.tile_pool(name="inp0", bufs=3) as inp0p,
            tc.tile_pool(name="inp1", bufs=2) as inp1p,
            tc.tile_pool(name="stg", bufs=3) as stgp,
            tc.tile_pool(name="gate", bufs=2) as gatep,
            tc.tile_pool(name="psum", bufs=1, space="PSUM") as psump,
        ):
            # ---- constants ----
            stT = constp.tile([D, NL * 4 * 2 * 2 * D], f8, tag="stT")
            nc.sync.dma_start(stT[:], st_d[:])

            bias = constp.tile([D, NL * 4], f32, tag="bias")
            nc.sync.dma_start(bias[:], bias_d[:])

            # ---- persistent state ----
            c_l = [statep.tile([D, CH * D], bf16, tag=f"c{l}", name=f"c{l}") for l in range(NL)]
            mp = statep.tile([D, CH * D], bf16, tag="mp")
            for l in range(NL):
                nc.vector.memset(c_l[l][:], 0.0)
            nc.vector.memset(mp[:], -1e30)

            def seg3(t):  # (p, s, c) view of an input tile
                return t[:].rearrange("p (s c) -> p s c", s=CH)

            def new_inp(pool, tag):
                t = pool.tile([D, NF], f8, tag=tag, name=tag)
                # zero the pad columns (0 and 257 of each chain segment)
                v = seg3(t)
                nc.vector.memset(v[:, :, 0:1], 0.0)
                nc.vector.memset(v[:, :, SEG - 1:SEG], 0.0)
                return t

            def load_dens(t_idx, dst):
                """DMA the step-t density into a contiguous staging tile,
                then gpsimd-scatter it into the strided x-slots.  A direct
                strided-dst DMA intermittently raced its completion
                semaphore (NaN poison at 64B burst boundaries)."""
                stg = stgp.tile([D, CH * D], f8, tag="xstg", name="xstg")
                nc.sync.dma_start(stg[:], dens_d[t_idx])
                nc.vector.tensor_copy(
                    seg3(dst)[:, :, 1:1 + D],
                    stg[:].rearrange("p (s r) -> p s r", s=CH),
                )

            GTAG = ["pf", "pi", "po", "pc"]

            def gates(l, inp):
                """DoubleRow gate matmuls + activations + cell update for
                layer l.  Returns (og, th) for the caller to route h."""
                i3 = seg3(inp)
                ps = [psump.tile([D, CH * D], f32, tag=GTAG[g], name=GTAG[g]) for g in range(4)]
                # issue order (f, i, c~, o): the cell-update DVE chain needs
                # f, i, c~ first; o is only needed for the final h product.
                for g in (0, 1, 3, 2):
                    for P in range(2):
                        idx = ((l * 4 + g) * 2 + P) * 2 * D
                        lhsT = stT[:, idx:idx + 2 * D].rearrange(
                            "p (two m) -> p two m", two=2)
                        for half in range(2):
                            rhs = i3[:, half * 4:(half + 1) * 4,
                                     2 * P:2 * P + 2 * D].rearrange(
                                "p s (j two) -> p two s j", two=2)
                            nc.tensor.matmul(
                                ps[g][:, half * 512:(half + 1) * 512],
                                lhsT, rhs,
                                start=(P == 0), stop=(P == 1),
                                perf_mode=DR,
                            )
                fg = gatep.tile([D, CH * D], bf16, tag="fg")
                ig = gatep.tile([D, CH * D], bf16, tag="ig")
                og = gatep.tile([D, CH * D], bf16, tag="og")
                cs = gatep.tile([D, CH * D], bf16, tag="cs")
                nc.scalar.activation(fg[:], ps[0][:], AF.Sigmoid,
                                     bias=bias[:, l * 4 + 0: l * 4 + 1])
                nc.scalar.activation(ig[:], ps[1][:], AF.Sigmoid,
                                     bias=bias[:, l * 4 + 1: l * 4 + 2])
                nc.scalar.activation(cs[:], ps[3][:], AF.Tanh,
                                     bias=bias[:, l * 4 + 3: l * 4 + 4])
                nc.scalar.activation(og[:], ps[2][:], AF.Sigmoid,
                                     bias=bias[:, l * 4 + 2: l * 4 + 3])
                t1 = gatep.tile([D, CH * D], bf16, tag="t1")
                t2 = gatep.tile([D, CH * D], bf16, tag="t2")
                nc.vector.tensor_mul(t1[:], fg[:], c_l[l][:])
                nc.vector.tensor_mul(t2[:], ig[:], cs[:])
                nc.vector.tensor_add(c_l[l][:], t1[:], t2[:])
                th = gatep.tile([D, CH * D], bf16, tag="th")
                nc.scalar.activation(th[:], c_l[l][:], AF.Tanh)
                return og, th

            # software-pipelined schedule with 1-step layer skew.  L1 input
            # tiles are triple-buffered and their density DMA is issued two
            # steps ahead so the transfer has ~2 full phases to land before
            # the matmuls read it.
            i0 = [new_inp(inp0p, "inp0"), new_inp(inp0p, "inp0")]
            nc.vector.memset(seg3(i0[0])[:, :, 129:129 + D], 0.0)  # h1_{-1}=0
            load_dens(0, i0[0])
            load_dens(1, i0[1])
            prevL2 = None                                 # L2 input, t-1

            for t in range(L):
                curL1 = i0[0]
                nxt1 = i0[1] if t + 1 < L else None
                if t + 2 < L:
                    t2 = new_inp(inp0p, "inp0")
                    load_dens(t + 2, t2)
                    i0 = [i0[1], t2]
                else:
                    i0 = [i0[1], None]

                l2in = new_inp(inp1p, "inp1")             # L2 input for step t
                if t == 0:
                    nc.vector.memset(seg3(l2in)[:, :, 129:129 + D], 0.0)

                # --- layer 1, step t ---
                og, th = gates(0, curL1)
                # h1_t -> h-part of L1 input t+1 (needed first, by L1 t+1's
                # matmuls) and x-part of L2 input t (consumed a phase later)
                if nxt1 is not None:
                    nc.vector.tensor_mul(seg3(nxt1)[:, :, 129:129 + D], og[:], th[:])
                nc.vector.tensor_mul(seg3(l2in)[:, :, 1:1 + D], og[:], th[:])

                # --- layer 2, step t-1 ---
                if t >= 1:
                    og, th = gates(1, prevL2)
                    nc.vector.tensor_mul(seg3(l2in)[:, :, 129:129 + D], og[:], th[:])
                    h2b = gatep.tile([D, CH * D], bf16, tag="h2b")
                    nc.vector.tensor_mul(h2b[:], og[:], th[:])
                    nc.vector.tensor_tensor(mp[:], mp[:], h2b[:], op=ALU.max)

                prevL2 = l2in

            # --- layer 2, final step L-1 ---
            og, th = gates(1, prevL2)
            h2b = gatep.tile([D, CH * D], bf16, tag="h2b")
            nc.vector.tensor_mul(h2b[:], og[:], th[:])
            nc.vector.tensor_tensor(mp[:], mp[:], h2b[:], op=ALU.max)

            nc.sync.dma_start(out_d[:], mp[:])

    nc.compile()
    return nc


def _prep_core_inputs(xe_y, st8, bias_arr, core, f8np):
    """xe_y: (B, 2, L, D) sqrt-normalized bf16 embeddings (axis1: 0=q, 1=a)."""
    sl = slice(4 * core, 4 * core + 4)
    # chains: s=0..3 -> q items, s=4..7 -> a items
    v = np.concatenate([xe_y[sl, 0], xe_y[sl, 1]], axis=0).astype(np.float32)  # (8, L, D)
    # density, laid out [t][w][s][r] so the per-step DMA is contiguous
    dens = np.einsum('stw,str->twsr', v, v).astype(f8np)       # (L, D, CH, D)
    return {"dens": dens, "st": st8, "bias": bias_arr}


def kernel(q, a, embed, conv_w, conv_b, lin_w, lin_b):
    import concourse.mybir as mybir
    from concourse import bass_utils
    import ml_dtypes

    bf16 = ml_dtypes.bfloat16
    f8np = mybir.dt.np(mybir.dt.float8e4)

    q = np.asarray(q); a = np.asarray(a)
    embed = np.asarray(embed, np.float32)
    conv_w = np.asarray(conv_w, np.float32)
    conv_b = np.asarray(conv_b, np.float32)
    lin_w = np.asarray(lin_w, np.float32)
    lin_b = np.asarray(lin_b, np.float32)

    # host: embedding gather + density normalization factors
    idx = np.stack([q, a], axis=1).astype(np.int64)            # (B, 2, L)
    xe = embed[idx].astype(np.float64)                         # (B, 2, L, D)
    dot = np.sum(xe * xe, axis=-1, keepdims=True) + 1e-4
    xe_y = (xe / np.sqrt(dot)).astype(np.float32).astype(bf16)

    # host: Toeplitz band stationaries  lhsT[(l,g,dh)] = B^T,
    # B[w, w'] = W[dh, w'-w+1]  (3 diagonals); packed in DoubleRow dh-pairs
    st = np.zeros((NL * 4 * 4, D, D), np.float32)
    for l in range(NL):
        for g in range(4):
            W = conv_w[l, g, 0, 0]                             # (4, 3)
            for dh in range(4):
                Bm = sum(W[dh, dw] * np.eye(D, k=dw - 1) for dw in range(3))
                st[(l * 4 + g) * 4 + dh] = Bm.T.astype(np.float32)
    # pack as one contiguous (D, 16*2*D) block: [w'][pair, plane, m]
    st8 = st.reshape(NL * 4 * 2, 2, D, D).transpose(2, 0, 1, 3).astype(f8np)
    st8 = np.ascontiguousarray(st8).reshape(D, NL * 4 * 2 * 2 * D)
    bias_arr = np.tile(conv_b.reshape(1, -1), (D, 1)).astype(np.float32)

    if "nc" not in _CACHE:
        _CACHE["nc"] = _build_nc()
    nc = _CACHE["nc"]

    in_maps = [_prep_core_inputs(xe_y, st8, bias_arr, i, f8np) for i in range(NCORES)]
    _CACHE["in_maps"] = in_maps
    res = bass_utils.run_bass_kernel_spmd(nc, in_maps, core_ids=list(range(NCORES)))

    # host: unshard + final linear + log_softmax
    q_p = np.zeros((B, D * D), np.float32)
    a_p = np.zeros((B, D * D), np.float32)
    for i in range(NCORES):
        out = np.asarray(res.results[i]["mp_out"], np.float32)  # (D w, CH*D)
        for s in range(CH):
            mp_T = out[:, s * D:(s + 1) * D]                   # (w, j)
            flat = np.ascontiguousarray(mp_T.T).reshape(-1)    # j-major
            if s < 4:
                q_p[4 * i + s] = flat
            else:
                a_p[4 * i + s - 4] = flat
    qa = np.concatenate([q_p, a_p], axis=1)
    score = qa @ lin_w.T + lin_b
    m = score.max(axis=1, keepdims=True)
    ls = score - m
    lse = np.log(np.exp(ls).sum(axis=1, keepdims=True))
    return (ls - lse).astype(np.float32)
